# revision 1
# baseline (speedup 1.0000x reference)
"""Trainium2 Bass kernel for nn_CorrelationLayer.

Math (derived from the reference conv formulation):
  out[b, 0, i, j] = sum_{c,y,x} feat1[b,c,y+i-2,x+j-2] * feat2[b,c,y,x]
with out-of-range feat1 reads contributing zero. i.e. 16 shifted
dot-products per batch over the (C, H, W) = (512, 4, 4) volume.

Strategy: pure data parallel over batch (8 cores x 512 batches).
Per core, batch goes on SBUF partitions (128 at a time -> perfectly
contiguous 4 MiB HBM reads), and each displacement's multiply+reduce
runs as fused DVE scalar_tensor_tensor (multiply + free-dim sum) per
valid y-row, with a final tensor_reduce folding the y-row partials.
"""

import sys

import numpy as np

sys.path.insert(0, "/opt/trn_rl_repo")

import concourse.bacc as bacc
import concourse.mybir as mybir
import concourse.tile as tile
from concourse import bass_utils

B, C, H, W = 4096, 512, 4, 4
NCORES = 8
BL = B // NCORES          # 512 batches per core
F = C * H * W             # 8192 elements per batch
PT = 128                  # partition tile (batches per SBUF tile)
NT = BL // PT             # 4 batch-tiles per core

_cached_nc = None


def _emit_body(nc, tc, f1d, f2d, outd):
    """Emit one full pass over this core's shard (inside a TileContext)."""
    with (
        tc.tile_pool(name="io", bufs=2) as iop,
        tc.tile_pool(name="sc", bufs=1) as scp,
        tc.tile_pool(name="ac", bufs=2) as acp,
    ):
        for t in range(NT):
            t1 = iop.tile([PT, F], mybir.dt.float32, tag="t1", name="t1")
            t2 = iop.tile([PT, F], mybir.dt.float32, tag="t2", name="t2")
            nc.sync.dma_start(out=t1[:], in_=f1d[t * PT:(t + 1) * PT, :])
            nc.sync.dma_start(out=t2[:], in_=f2d[t * PT:(t + 1) * PT, :])
            prod = scp.tile([PT, F], mybir.dt.float32, tag="prod", name="prod")
            # per-(displacement, y-row) partial sums, padded to 4 rows
            acc = acp.tile([PT, 64], mybir.dt.float32, tag="acc", name="acc")
            fin = acp.tile([PT, 16], mybir.dt.float32, tag="fin", name="fin")
            nc.vector.memset(acc[:], 0.0)

            a1 = t1.rearrange("p (c y x) -> p c y x", y=H, x=W)
            a2 = t2.rearrange("p (c y x) -> p c y x", y=H, x=W)
            ap = prod.rearrange("p (c y x) -> p c y x", y=H, x=W)

            # Walrus only accepts 2 free dims on DVE ops, so each
            # displacement is split into its h y-rows ([c, x-window] APs);
            # scalar_tensor_tensor fuses multiply + free-dim reduce.
            for i in range(4):
                for j in range(4):
                    di, dj = i - 2, j - 2
                    y0, y1 = max(0, -di), min(H - 1, H - 1 - di)
                    x0, x1 = max(0, -dj), min(W - 1, W - 1 - dj)
                    for y in range(y0, y1 + 1):
                        w1 = a1[:, :, y + di, x0 + dj:x1 + 1 + dj]
                        w2 = a2[:, :, y, x0:x1 + 1]
                        po = ap[:, :, y, x0:x1 + 1]
                        s = (i * 4 + j) * 4 + (y - y0)
                        nc.vector.scalar_tensor_tensor(
                            out=po,
                            in0=w1,
                            scalar=1.0,
                            in1=w2,
                            op0=mybir.AluOpType.mult,
                            op1=mybir.AluOpType.mult,
                            accum_out=acc[:, s:s + 1],
                        )
            nc.vector.tensor_reduce(
                out=fin[:],
                in_=acc.rearrange("p (d y) -> p d y", y=4),
                axis=mybir.AxisListType.X,
                op=mybir.AluOpType.add,
            )
            nc.sync.dma_start(out=outd[t * PT:(t + 1) * PT, :], in_=fin[:])


def _build(reps: int = 1):
    nc = bacc.Bacc("TRN2", target_bir_lowering=False, debug=False)
    f1d = nc.dram_tensor("feat1", [BL, F], mybir.dt.float32, kind="ExternalInput").ap()
    f2d = nc.dram_tensor("feat2", [BL, F], mybir.dt.float32, kind="ExternalInput").ap()
    outd = nc.dram_tensor("out", [BL, 16], mybir.dt.float32, kind="ExternalOutput").ap()

    with tile.TileContext(nc) as tc:
        if reps == 1:
            _emit_body(nc, tc, f1d, f2d, outd)
        else:
            with tc.For_i(0, reps, 1):
                _emit_body(nc, tc, f1d, f2d, outd)

    nc.compile()
    return nc


def _get_nc():
    global _cached_nc
    if _cached_nc is None:
        _cached_nc = _build()
    return _cached_nc


def kernel(feat1, feat2):
    f1 = np.ascontiguousarray(np.asarray(feat1, dtype=np.float32)).reshape(B, F)
    f2 = np.ascontiguousarray(np.asarray(feat2, dtype=np.float32)).reshape(B, F)
    nc = _get_nc()
    in_maps = [
        {"feat1": f1[k * BL:(k + 1) * BL], "feat2": f2[k * BL:(k + 1) * BL]}
        for k in range(NCORES)
    ]
    res = bass_utils.run_bass_kernel_spmd(nc, in_maps, list(range(NCORES)))
    out = np.concatenate([res.results[k]["out"] for k in range(NCORES)], axis=0)
    return out.reshape(B, 1, H, W)



# revision 5
# speedup vs baseline: 1.2099x; 1.2099x over previous
"""Trainium2 Bass kernel for nn_CorrelationLayer.

Math (derived from the reference conv formulation):
  out[b, 0, i, j] = sum_{c,y,x} feat1[b,c,y+i-2,x+j-2] * feat2[b,c,y,x]
with out-of-range feat1 reads contributing zero. i.e. 16 shifted
dot-products per batch over the (C, H, W) = (512, 4, 4) volume.

Strategy: pure data parallel over batch (8 cores x 512 batches), batch
on SBUF partitions (128 at a time). The multiply+reduce work is split
between the DVE and the Activation engine (the only two engines with
independent SBUF ports -- gpsimd shares the DVE ports and only steals
bandwidth; no fused DVE op has a 2-byte fast mode, but plain
tensor_tensor in bf16 runs at 2 elem/cycle):
  - inputs stream through a small fp32 staging ring and are converted
    once to bf16 (split DVE/Act to balance engine load);
  - lane A (DVE): fused scalar_tensor_tensor (multiply + accumulate)
    per (displacement, y-row) on the 7 border displacements;
  - lane B (DVE+Act): bf16 tensor_tensor at 2x writes each large
    displacement's product to a bf16 scratch; the Act engine reduces it
    with one Copy+accum_out pass (1.2 GHz, own SBUF port). Covers the 9
    center displacements (~70% of the element volume).
"""

import sys

import numpy as np

sys.path.insert(0, "/opt/trn_rl_repo")

import concourse.bacc as bacc
import concourse.mybir as mybir
import concourse.tile as tile
from concourse import bass_utils

B, C, H, W = 4096, 512, 4, 4
NCORES = 8
BL = B // NCORES          # 512 batches per core
F = C * H * W             # 8192 elements per batch
PT = 128                  # partition tile (batches per SBUF tile)
NT = BL // PT             # 4 batch-tiles per core
HF = F // 2               # staging half-tile

# displacements on the bf16 DVE-mult + Act-reduce lane (largest windows)
B_SET = {(1, 1), (1, 2), (1, 3), (2, 1), (2, 2), (2, 3), (3, 1), (3, 2),
         (3, 3)}

_cached_nc = None


def _windows(i, j):
    """Valid y/x ranges for displacement (i-2, j-2)."""
    di, dj = i - 2, j - 2
    y0, y1 = max(0, -di), min(H - 1, H - 1 - di)
    x0, x1 = max(0, -dj), min(W - 1, W - 1 - dj)
    return di, dj, y0, y1, x0, x1


def _emit_body(nc, tc, f1d, f2d, outd):
    """Emit one full pass over this core's shard (inside a TileContext)."""
    with (
        tc.tile_pool(name="stg", bufs=4) as stp,
        tc.tile_pool(name="bf", bufs=2) as bfp,
        tc.tile_pool(name="scr", bufs=2) as scp,
        tc.tile_pool(name="pr", bufs=1) as prp,
        tc.tile_pool(name="dmp", bufs=1) as dmp,
        tc.tile_pool(name="ac", bufs=2) as acp,
    ):
        dummy = dmp.tile([PT, 8192], mybir.dt.bfloat16, tag="dummy", name="dummy")
        for t in range(NT):
            b1 = bfp.tile([PT, F], mybir.dt.bfloat16, tag="b1", name="b1")
            b2 = bfp.tile([PT, F], mybir.dt.bfloat16, tag="b2", name="b2")
            # stream each tensor through fp32 staging halves -> bf16.
            # DVE converts the first half of feat1 (tensor_copy runs at
            # 2 elem/cycle even fp32-in); Act converts the rest.
            for src, dst, name in ((f1d, b1, "s1"), (f2d, b2, "s2")):
                for h in range(2):
                    stg = stp.tile([PT, HF], mybir.dt.float32, tag="stg",
                                   name=f"stg_{t}_{name}_{h}")
                    nc.sync.dma_start(
                        out=stg[:],
                        in_=src[t * PT:(t + 1) * PT, h * HF:(h + 1) * HF])
                    if name == "s1" and h == 0:
                        nc.vector.tensor_copy(out=dst[:, 0:HF], in_=stg[:])
                    else:
                        nc.scalar.activation(
                            out=dst[:, h * HF:(h + 1) * HF], in_=stg[:],
                            func=mybir.ActivationFunctionType.Copy)

            prod = prp.tile([PT, 2048], mybir.dt.bfloat16, tag="prod",
                            name="prod")
            accA = acp.tile([PT, 32], mybir.dt.float32, tag="accA", name="accA")
            accB = acp.tile([PT, 16], mybir.dt.float32, tag="accB", name="accB")
            finA = acp.tile([PT, 16], mybir.dt.float32, tag="finA", name="finA")
            fin = acp.tile([PT, 16], mybir.dt.float32, tag="fin", name="fin")
            nc.vector.memset(accA[:], 0.0)
            nc.vector.memset(accB[:], 0.0)

            g1 = b1.rearrange("p (c y x) -> p c y x", y=H, x=W)
            g2 = b2.rearrange("p (c y x) -> p c y x", y=H, x=W)
            ap = prod.rearrange("p (c x) -> p c x", x=4)

            # ---- lane A: DVE fused STT (bf16 in, fp32 accum) ----
            acol = 0
            a_cols = {}
            for i in range(4):
                for j in range(4):
                    if (i, j) in B_SET:
                        continue
                    di, dj, y0, y1, x0, x1 = _windows(i, j)
                    nx = x1 - x0 + 1
                    a_cols[(i, j)] = (acol, y1 - y0 + 1)
                    for y in range(y0, y1 + 1):
                        w1 = g1[:, :, y + di, x0 + dj:x1 + 1 + dj]
                        w2 = g2[:, :, y, x0:x1 + 1]
                        po = ap[:, :, 0:nx]
                        nc.vector.scalar_tensor_tensor(
                            out=po, in0=w1, scalar=1.0, in1=w2,
                            op0=mybir.AluOpType.mult,
                            op1=mybir.AluOpType.mult,
                            accum_out=accA[:, acol:acol + 1])
                        acol += 1

            # ---- lane B: DVE bf16 TT mult into scratch, Act accum-reduce
            for (i, j) in sorted(B_SET):
                di, dj, y0, y1, x0, x1 = _windows(i, j)
                ny, nx = y1 - y0 + 1, x1 - x0 + 1
                scr = scp.tile([PT, C * 16], mybir.dt.bfloat16, tag="scr",
                               name=f"scr_{t}_{i}{j}")
                if dj == 0:
                    # full-width x: y rows contiguous, single fat TT
                    w1 = g1[:, :, y0 + di:y1 + 1 + di, :].rearrange(
                        "p c y x -> p c (y x)")
                    w2 = g2[:, :, y0:y1 + 1, :].rearrange("p c y x -> p c (y x)")
                    po = scr[:, 0:C * ny * 4].rearrange("p (c f) -> p c f",
                                                        f=ny * 4)
                    nc.vector.tensor_tensor(out=po, in0=w1, in1=w2,
                                            op=mybir.AluOpType.mult)
                else:
                    for y in range(y0, y1 + 1):
                        w1 = g1[:, :, y + di, x0 + dj:x1 + 1 + dj]
                        w2 = g2[:, :, y, x0:x1 + 1]
                        seg = (y - y0) * C * nx
                        po = scr[:, seg:seg + C * nx].rearrange(
                            "p (c f) -> p c f", f=nx)
                        nc.vector.tensor_tensor(out=po, in0=w1, in1=w2,
                                                op=mybir.AluOpType.mult)
                n = C * ny * nx
                nc.scalar.activation(
                    out=dummy[:, 0:n], in_=scr[:, 0:n],
                    func=mybir.ActivationFunctionType.Copy,
                    accum_out=accB[:, i * 4 + j:i * 4 + j + 1])

            # ---- combine accA rows into finA columns, add accB ----
            nc.vector.memset(finA[:], 0.0)
            for (i, j), (c0, ny) in a_cols.items():
                nc.vector.tensor_reduce(
                    out=finA[:, i * 4 + j:i * 4 + j + 1],
                    in_=accA[:, c0:c0 + ny],
                    axis=mybir.AxisListType.X,
                    op=mybir.AluOpType.add,
                )
            nc.vector.tensor_tensor(out=fin[:], in0=finA[:], in1=accB[:],
                                    op=mybir.AluOpType.add)
            nc.sync.dma_start(out=outd[t * PT:(t + 1) * PT, :], in_=fin[:])


def _build(reps: int = 1):
    nc = bacc.Bacc("TRN2", target_bir_lowering=False, debug=False)
    f1d = nc.dram_tensor("feat1", [BL, F], mybir.dt.float32, kind="ExternalInput").ap()
    f2d = nc.dram_tensor("feat2", [BL, F], mybir.dt.float32, kind="ExternalInput").ap()
    outd = nc.dram_tensor("out", [BL, 16], mybir.dt.float32, kind="ExternalOutput").ap()

    with tile.TileContext(nc) as tc:
        if reps == 1:
            _emit_body(nc, tc, f1d, f2d, outd)
        else:
            with tc.For_i(0, reps, 1):
                _emit_body(nc, tc, f1d, f2d, outd)

    nc.compile()
    return nc


def _get_nc():
    global _cached_nc
    if _cached_nc is None:
        _cached_nc = _build()
    return _cached_nc


def kernel(feat1, feat2):
    f1 = np.ascontiguousarray(np.asarray(feat1, dtype=np.float32)).reshape(B, F)
    f2 = np.ascontiguousarray(np.asarray(feat2, dtype=np.float32)).reshape(B, F)
    nc = _get_nc()
    in_maps = [
        {"feat1": f1[k * BL:(k + 1) * BL], "feat2": f2[k * BL:(k + 1) * BL]}
        for k in range(NCORES)
    ]
    res = bass_utils.run_bass_kernel_spmd(nc, in_maps, list(range(NCORES)))
    out = np.concatenate([res.results[k]["out"] for k in range(NCORES)], axis=0)
    return out.reshape(B, 1, H, W)
